# revision 4
# baseline (speedup 1.0000x reference)
"""Trainium2 Bass kernel for nn_Attention_72404558676364.

Math: the reference computes
    pre[l,b,:] = hs_encoder[l,b,:] @ We.T + (hidden @ Wh.T + b_att)[b,:]
    attn[b,l]  = pre[l,b,:] . v
    out        = softmax(attn, axis=l)
Softmax over l is shift-invariant, so the hidden/Wh/b_att term (constant in
l for fixed b) cancels exactly and the einsum collapses to a single matvec:
    attn[b,l] = hs_encoder[l,b,:] . w_eff,   w_eff = We.T @ v
The device does one pass over hs_encoder plus the small We.T @ v, then a
per-batch softmax.

The kernel is DMA-bound (hs_encoder must cross HBM->SBUF once), so the wire
format is fp16: logits carry ~1e-2 absolute noise which softmax largely
cancels (measured end-to-end rel err 1.8e-3 vs the 2e-2 gate).  PE matmuls
run fp16 at full rate (1 col/cycle vs fp32's 1/4), so the tensor engine
stays ahead of the DMA stream and tile buffers recycle without stalls.

Sharding: data-parallel over batch; core c handles batches [8c, 8c+8).
hs_encoder shards are pre-transposed on the host to [H, Bc*L] so every DMA
is contiguous per partition.

w_eff is computed on-device as w_cols[p,hc] directly (lhsT = We 128x128
tile, rhs = v chunk): output lands transposed in one PSUM bank, no PE
transpose pass needed.  DMA order: We chunks first (both rings), then hs
tiles grouped (4,3,1) batch-major so softmax chains pipeline behind the
matmul stream and only the final single-batch chain is exposed as tail.
"""

import sys

import numpy as np

for _p in (
    "/root/.axon_site",
    "/root/.axon_site/_ro/trn_rl_repo",
    "/root/.axon_site/_ro/pypackages",
):
    if _p not in sys.path:
        sys.path.append(_p)

import concourse.bass as bass
import concourse.mybir as mybir
import concourse.tile as tile
from concourse.bass_utils import run_bass_kernel_spmd

H = 1024
L = 512
B = 64
NCORES = 8
BC = B // NCORES  # batches per core
P = 128
HC = H // P  # 128-wide chunks of the contraction dim

F32 = mybir.dt.float32
F16 = mybir.dt.float16

_split_n = 0


def _split_multi_waits(nc):
    """Hoist extra sem waits onto same-engine NOPs.

    The walrus build in this container rejects any instruction carrying more
    than one sync-wait ("Too many sync wait commands"), but Tile emits
    multi-wait instructions whenever one op depends on several producers.
    A NOP on the same engine immediately before the instruction waits
    equivalently (per-engine program order).
    """
    global _split_n
    engines = [
        mybir.EngineType.SP,
        mybir.EngineType.Activation,
        mybir.EngineType.DVE,
        mybir.EngineType.PE,
        mybir.EngineType.Pool,
    ]
    for fn in nc.m.functions:
        for blk in fn.blocks:
            new_insts = []
            for inst in blk.instructions:
                si = getattr(inst, "sync_info", None)
                if si is not None and si.on_wait and len(si.on_wait) > 1:
                    waits = list(si.on_wait)
                    si.on_wait = waits[:1]
                    # The exit drain carries one wait per DMA queue sem; its
                    # waits may run on ANY engine because the all-engine
                    # barrier right after it orders everything.  Mid-kernel
                    # instructions need same-engine NOPs (program order).
                    wide = (
                        isinstance(inst, mybir.InstDrain) and len(waits) > 3
                    )
                    for k, w in enumerate(waits[1:]):
                        _split_n += 1
                        eng = engines[k % len(engines)] if wide else inst.engine
                        new_insts.append(
                            mybir.InstNoOp(
                                name=f"I-wsplit-{_split_n}",
                                engine=eng,
                                sync_info=mybir.SyncInfo(
                                    on_wait=[w], on_update=[]
                                ),
                                bass_nofuse=True,
                            )
                        )
                new_insts.append(inst)
            blk.instructions = new_insts


def _build():
    nc = bass.Bass(target_bir_lowering=False, enable_partition_id=False)
    hsT = nc.dram_tensor("hsT", [H, BC * L], F16, kind="ExternalInput")
    we = nc.dram_tensor("We", [H, H], F16, kind="ExternalInput")
    v = nc.dram_tensor("v", [P, HC], F16, kind="ExternalInput")
    out = nc.dram_tensor("out", [BC, L], F32, kind="ExternalOutput")

    with tile.TileContext(nc) as tc:
        with (
            tc.tile_pool(name="singles", bufs=1) as singles,
            tc.tile_pool(name="hs", bufs=8) as hs_pool,
            tc.tile_pool(name="srow", bufs=5) as srow_pool,
            tc.tile_pool(name="psw", bufs=1, space="PSUM") as psw_pool,
            tc.tile_pool(name="pss", bufs=2, space="PSUM") as pss_pool,
        ):
            # ---- small operands ---------------------------------------
            v_sb = singles.tile([P, HC], F16)
            nc.sync.dma_start(out=v_sb[:], in_=v[:])

            # Per-chunk We DMAs alternating between the two HWDGE rings.
            we_sb = singles.tile([P, HC, H], F16)
            for kc in range(HC):
                eng = nc.sync if kc % 2 == 0 else nc.scalar
                eng.dma_start(
                    out=we_sb[:, kc, :], in_=we[kc * P : (kc + 1) * P, :]
                )

            # ---- w_cols[p, hc] = w_eff[hc*128+p] ----------------------
            # lhsT = We 128x128 tile (k-chunk rows, h-slice cols), rhs = v
            # k-chunk [128,1].  The result lands already "transposed" as
            # [128, HC] in one PSUM bank: no PE transpose pass.  hc must be
            # the OUTER loop: PSUM accumulation-group state is per PE
            # column group, so only one group may be open at a time here
            # (kc-outer interleaving returns garbage on HW).
            psw = psw_pool.tile([P, HC], F32)
            for hc in range(HC):
                for kc in range(HC):
                    nc.tensor.matmul(
                        psw[:, hc : hc + 1],
                        lhsT=we_sb[:, kc, hc * P : (hc + 1) * P],
                        rhs=v_sb[:, kc : kc + 1],
                        start=(kc == 0),
                        stop=(kc == HC - 1),
                    )
            w16 = singles.tile([P, HC], F16)
            nc.scalar.copy(out=w16[:], in_=psw[:])

            # ---- scores[j, l] = hsT[:, j*L+l] . w_eff ------------------
            # Batch-major groups.  A batch's scores close only when its
            # group's LAST h-chunk lands (closure is DMA-paced), so groups
            # must be small enough that each closure's softmax chains
            # (~2.3us each on DVE/ACT) finish inside the next group's DMA
            # window (~3.4us/MB).  (2,2,2,1,1) staggers closures every
            # ~3.4us and leaves only the final single-batch chain exposed.
            groups = [(0, 2), (2, 2), (4, 2), (6, 1), (7, 1)]
            for gi, (j0, ng) in enumerate(groups):
                tiles = []
                for hc in range(HC):
                    eng = nc.sync if hc % 2 == 0 else nc.scalar
                    t = hs_pool.tile([P, ng * L], F16, tag=f"hs{ng}")
                    eng.dma_start(
                        out=t[:],
                        in_=hsT[
                            hc * P : (hc + 1) * P, j0 * L : (j0 + ng) * L
                        ],
                    )
                    tiles.append(t)
                ps = pss_pool.tile([P, L], F32, tag="pss")
                if ng == 1:
                    # fp16 matmuls are cheap (512 cols ~ 280ns): plain
                    # sequential accumulation leaves only the last chunk's
                    # matmul + one softmax chain exposed after the final
                    # DMA.
                    for hc in range(HC):
                        nc.tensor.matmul(
                            ps[0:1, :],
                            lhsT=w16[:, hc : hc + 1],
                            rhs=tiles[hc][:, 0:L],
                            start=(hc == 0),
                            stop=(hc == HC - 1),
                        )
                else:
                    # Skewed wavefront: batch g's accumulation closes g
                    # steps early, so its softmax chain overlaps the
                    # remaining batches' matmuls.
                    for step in range(HC + ng - 1):
                        for g in range(ng):
                            hc = step - g
                            if not 0 <= hc < HC:
                                continue
                            nc.tensor.matmul(
                                ps[32 * g : 32 * g + 1, :],
                                lhsT=w16[:, hc : hc + 1],
                                rhs=tiles[hc][:, g * L : (g + 1) * L],
                                start=(hc == 0),
                                stop=(hc == HC - 1),
                                tile_position=(0, 32 * g),
                            )
                for g in range(ng):
                    j = j0 + g
                    # Per-batch softmax on idle DVE/ACT while later batches'
                    # matmuls stream, reading scores straight from PSUM.
                    row = ps[32 * g : 32 * g + 1, :]
                    negmax = srow_pool.tile([1, 1], F32)
                    nc.vector.reduce_max(
                        out=negmax[:], in_=row, axis=mybir.AxisListType.X,
                        negate=True,
                    )
                    exps = srow_pool.tile([1, L], F32)
                    sums = srow_pool.tile([1, 1], F32)
                    nc.scalar.activation(
                        out=exps[:],
                        in_=row,
                        func=mybir.ActivationFunctionType.Exp,
                        bias=negmax[:],
                        scale=1.0,
                        accum_out=sums[:],
                    )
                    rsum = srow_pool.tile([1, 1], F32)
                    nc.vector.reciprocal(out=rsum[:], in_=sums[:])
                    orow = srow_pool.tile([1, L], F32)
                    nc.vector.tensor_scalar_mul(
                        out=orow[:], in0=exps[:], scalar1=rsum[:]
                    )
                    if gi == len(groups) - 1:
                        # rings are idle at the tail; HWDGE has the lower
                        # first-byte latency
                        nc.sync.dma_start(out=out[j : j + 1, :], in_=orow[:])
                    else:
                        # SWDGE keeps mid-stream stores off the HWDGE rings
                        # so their waits never stall the input DMAs.
                        nc.gpsimd.dma_start(out=out[j : j + 1, :], in_=orow[:])

    _split_multi_waits(nc)
    return nc


_NC_CACHE = None


def _make_in_maps(hs_encoder, W_att, vector):
    hs_encoder = np.asarray(hs_encoder, dtype=np.float32)
    we_np = np.ascontiguousarray(W_att[:, H:], dtype=np.float16)
    v_np = np.ascontiguousarray(
        np.asarray(vector, dtype=np.float32)[:, 0].reshape(HC, P).T,
        dtype=np.float16,
    )

    in_maps = []
    for c in range(NCORES):
        shard = hs_encoder[:, c * BC : (c + 1) * BC, :]  # [L, BC, H]
        hst = np.ascontiguousarray(
            shard.transpose(2, 1, 0).reshape(H, BC * L), dtype=np.float16
        )
        in_maps.append({"hsT": hst, "We": we_np, "v": v_np})
    return in_maps


def kernel(hidden, hs_encoder, W_att, b_att, vector):
    global _NC_CACHE
    if _NC_CACHE is None:
        _NC_CACHE = _build()
    nc = _NC_CACHE

    in_maps = _make_in_maps(hs_encoder, W_att, vector)
    res = run_bass_kernel_spmd(nc, in_maps, core_ids=list(range(NCORES)))
    out = np.concatenate([res.results[c]["out"] for c in range(NCORES)], axis=0)
    return out[:, None, :].astype(np.float32)


# revision 6
# speedup vs baseline: 1.0539x; 1.0539x over previous
"""Trainium2 Bass kernel for nn_Attention_72404558676364.

Math: the reference computes
    pre[l,b,:] = hs_encoder[l,b,:] @ We.T + (hidden @ Wh.T + b_att)[b,:]
    attn[b,l]  = pre[l,b,:] . v
    out        = softmax(attn, axis=l)
Softmax over l is shift-invariant, so the hidden/Wh/b_att term (constant in
l for fixed b) cancels exactly and the einsum collapses to a single matvec:
    attn[b,l] = hs_encoder[l,b,:] . w_eff,   w_eff = We.T @ v
The device does one pass over hs_encoder plus the small We.T @ v, then a
per-batch softmax.

The kernel is DMA-bound (hs_encoder must cross HBM->SBUF once), so the wire
format is fp16: logits carry ~1e-2 absolute noise which softmax largely
cancels (measured end-to-end rel err 1.8e-3 vs the 2e-2 gate).  PE matmuls
run fp16 at full rate (1 col/cycle vs fp32's 1/4), so the tensor engine
stays ahead of the DMA stream and tile buffers recycle without stalls.

Sharding: data-parallel over batch; core c handles batches [8c, 8c+8).
hs_encoder shards are pre-transposed on the host to [H, Bc*L] so every DMA
is contiguous per partition.

w_eff is computed on-device as w_cols[p,hc] directly (lhsT = We 128x128
tile, rhs = v chunk): output lands transposed in one PSUM bank, no PE
transpose pass needed.  DMA order: We chunks first (both rings), then hs
tiles grouped (4,3,1) batch-major so softmax chains pipeline behind the
matmul stream and only the final single-batch chain is exposed as tail.
"""

import sys

import numpy as np

for _p in (
    "/root/.axon_site",
    "/root/.axon_site/_ro/trn_rl_repo",
    "/root/.axon_site/_ro/pypackages",
):
    if _p not in sys.path:
        sys.path.append(_p)

import concourse.bass as bass
import concourse.mybir as mybir
import concourse.tile as tile
from concourse.bass_utils import run_bass_kernel_spmd

H = 1024
L = 512
B = 64
NCORES = 8
BC = B // NCORES  # batches per core
P = 128
HC = H // P  # 128-wide chunks of the contraction dim

F32 = mybir.dt.float32
F16 = mybir.dt.float16

_split_n = 0


def _split_multi_waits(nc):
    """Hoist extra sem waits onto same-engine NOPs.

    The walrus build in this container rejects any instruction carrying more
    than one sync-wait ("Too many sync wait commands"), but Tile emits
    multi-wait instructions whenever one op depends on several producers.
    A NOP on the same engine immediately before the instruction waits
    equivalently (per-engine program order).
    """
    global _split_n
    engines = [
        mybir.EngineType.SP,
        mybir.EngineType.Activation,
        mybir.EngineType.DVE,
        mybir.EngineType.PE,
        mybir.EngineType.Pool,
    ]
    for fn in nc.m.functions:
        for blk in fn.blocks:
            new_insts = []
            for inst in blk.instructions:
                si = getattr(inst, "sync_info", None)
                if si is not None and si.on_wait and len(si.on_wait) > 1:
                    waits = list(si.on_wait)
                    si.on_wait = waits[:1]
                    # The exit drain carries one wait per DMA queue sem; its
                    # waits may run on ANY engine because the all-engine
                    # barrier right after it orders everything.  Mid-kernel
                    # instructions need same-engine NOPs (program order).
                    wide = (
                        isinstance(inst, mybir.InstDrain) and len(waits) > 3
                    )
                    for k, w in enumerate(waits[1:]):
                        _split_n += 1
                        eng = engines[k % len(engines)] if wide else inst.engine
                        new_insts.append(
                            mybir.InstNoOp(
                                name=f"I-wsplit-{_split_n}",
                                engine=eng,
                                sync_info=mybir.SyncInfo(
                                    on_wait=[w], on_update=[]
                                ),
                                bass_nofuse=True,
                            )
                        )
                new_insts.append(inst)
            blk.instructions = new_insts


def _build():
    nc = bass.Bass(target_bir_lowering=False, enable_partition_id=False)
    hsT = nc.dram_tensor("hsT", [H, BC * L], F16, kind="ExternalInput")
    we = nc.dram_tensor("We", [H, H], F16, kind="ExternalInput")
    v = nc.dram_tensor("v", [P, HC], F16, kind="ExternalInput")
    out = nc.dram_tensor("out", [BC, L], F32, kind="ExternalOutput")

    with tile.TileContext(nc) as tc:
        with (
            tc.tile_pool(name="singles", bufs=1) as singles,
            tc.tile_pool(name="hs", bufs=8) as hs_pool,
            tc.tile_pool(name="srow", bufs=5) as srow_pool,
            tc.tile_pool(name="psw", bufs=1, space="PSUM") as psw_pool,
            tc.tile_pool(name="pss", bufs=3, space="PSUM") as pss_pool,
        ):
            # ---- small operands ---------------------------------------
            v_sb = singles.tile([P, HC], F16)
            nc.sync.dma_start(out=v_sb[:], in_=v[:])

            # Per-chunk We DMAs alternating between the two HWDGE rings.
            we_sb = singles.tile([P, HC, H], F16)
            for kc in range(HC):
                eng = nc.sync if kc % 2 == 0 else nc.scalar
                eng.dma_start(
                    out=we_sb[:, kc, :], in_=we[kc * P : (kc + 1) * P, :]
                )

            # ---- w_cols[p, hc] = w_eff[hc*128+p] ----------------------
            # lhsT = We 128x128 tile (k-chunk rows, h-slice cols), rhs = v
            # k-chunk [128,1].  The result lands already "transposed" as
            # [128, HC] in one PSUM bank: no PE transpose pass.  hc must be
            # the OUTER loop: PSUM accumulation-group state is per PE
            # column group, so only one group may be open at a time here
            # (kc-outer interleaving returns garbage on HW).
            psw = psw_pool.tile([P, HC], F32)
            for hc in range(HC):
                for kc in range(HC):
                    nc.tensor.matmul(
                        psw[:, hc : hc + 1],
                        lhsT=we_sb[:, kc, hc * P : (hc + 1) * P],
                        rhs=v_sb[:, kc : kc + 1],
                        start=(kc == 0),
                        stop=(kc == HC - 1),
                    )
            w16 = singles.tile([P, HC], F16)
            nc.scalar.copy(out=w16[:], in_=psw[:])

            # ---- scores[j, l] = hsT[:, j*L+l] . w_eff ------------------
            # Batch-major groups.  A batch's scores close only when its
            # group's LAST h-chunk lands (closure is DMA-paced), so groups
            # must be small enough that each closure's softmax chains
            # (~2.3us each on DVE/ACT) finish inside the next group's DMA
            # window (~3.4us/MB).  (2,2,2,1,1) staggers closures every
            # ~3.4us and leaves only the final single-batch chain exposed.
            groups = [(0, 2), (2, 2), (4, 2), (6, 1), (7, 1)]
            for gi, (j0, ng) in enumerate(groups):
                tiles = []
                for hc in range(HC):
                    eng = nc.sync if hc % 2 == 0 else nc.scalar
                    # Unique tag per group: everything fits in SBUF, so any
                    # buffer reuse would serialize a later group's DMA
                    # behind an earlier group's matmuls for nothing.
                    t = hs_pool.tile([P, ng * L], F16, tag=f"hs{gi}")
                    eng.dma_start(
                        out=t[:],
                        in_=hsT[
                            hc * P : (hc + 1) * P, j0 * L : (j0 + ng) * L
                        ],
                    )
                    tiles.append(t)
                ps = pss_pool.tile([P, L], F32, tag="pss")
                if ng == 1:
                    # fp16 matmuls are cheap (512 cols ~ 280ns): plain
                    # sequential accumulation leaves only the last chunk's
                    # matmul + one softmax chain exposed after the final
                    # DMA.
                    for hc in range(HC):
                        nc.tensor.matmul(
                            ps[0:1, :],
                            lhsT=w16[:, hc : hc + 1],
                            rhs=tiles[hc][:, 0:L],
                            start=(hc == 0),
                            stop=(hc == HC - 1),
                        )
                else:
                    # Skewed wavefront: batch g's accumulation closes g
                    # steps early, so its softmax chain overlaps the
                    # remaining batches' matmuls.
                    for step in range(HC + ng - 1):
                        for g in range(ng):
                            hc = step - g
                            if not 0 <= hc < HC:
                                continue
                            nc.tensor.matmul(
                                ps[32 * g : 32 * g + 1, :],
                                lhsT=w16[:, hc : hc + 1],
                                rhs=tiles[hc][:, g * L : (g + 1) * L],
                                start=(hc == 0),
                                stop=(hc == HC - 1),
                                tile_position=(0, 32 * g),
                            )
                for g in range(ng):
                    j = j0 + g
                    # Per-batch softmax on idle DVE/ACT while later batches'
                    # matmuls stream, reading scores straight from PSUM.
                    row = ps[32 * g : 32 * g + 1, :]
                    negmax = srow_pool.tile([1, 1], F32)
                    nc.vector.reduce_max(
                        out=negmax[:], in_=row, axis=mybir.AxisListType.X,
                        negate=True,
                    )
                    exps = srow_pool.tile([1, L], F32)
                    sums = srow_pool.tile([1, 1], F32)
                    nc.scalar.activation(
                        out=exps[:],
                        in_=row,
                        func=mybir.ActivationFunctionType.Exp,
                        bias=negmax[:],
                        scale=1.0,
                        accum_out=sums[:],
                    )
                    rsum = srow_pool.tile([1, 1], F32)
                    nc.vector.reciprocal(out=rsum[:], in_=sums[:])
                    orow = srow_pool.tile([1, L], F32)
                    nc.vector.tensor_scalar_mul(
                        out=orow[:], in0=exps[:], scalar1=rsum[:]
                    )
                    if gi == len(groups) - 1:
                        # rings are idle at the tail; HWDGE has the lower
                        # first-byte latency
                        nc.sync.dma_start(out=out[j : j + 1, :], in_=orow[:])
                    else:
                        # SWDGE keeps mid-stream stores off the HWDGE rings
                        # so their waits never stall the input DMAs.
                        nc.gpsimd.dma_start(out=out[j : j + 1, :], in_=orow[:])

    _split_multi_waits(nc)
    return nc


_NC_CACHE = None


def _make_in_maps(hs_encoder, W_att, vector):
    hs_encoder = np.asarray(hs_encoder, dtype=np.float32)
    we_np = np.ascontiguousarray(W_att[:, H:], dtype=np.float16)
    v_np = np.ascontiguousarray(
        np.asarray(vector, dtype=np.float32)[:, 0].reshape(HC, P).T,
        dtype=np.float16,
    )

    in_maps = []
    for c in range(NCORES):
        shard = hs_encoder[:, c * BC : (c + 1) * BC, :]  # [L, BC, H]
        hst = np.ascontiguousarray(
            shard.transpose(2, 1, 0).reshape(H, BC * L), dtype=np.float16
        )
        in_maps.append({"hsT": hst, "We": we_np, "v": v_np})
    return in_maps


def kernel(hidden, hs_encoder, W_att, b_att, vector):
    global _NC_CACHE
    if _NC_CACHE is None:
        _NC_CACHE = _build()
    nc = _NC_CACHE

    in_maps = _make_in_maps(hs_encoder, W_att, vector)
    res = run_bass_kernel_spmd(nc, in_maps, core_ids=list(range(NCORES)))
    out = np.concatenate([res.results[c]["out"] for c in range(NCORES)], axis=0)
    return out[:, None, :].astype(np.float32)
